# revision 15
# baseline (speedup 1.0000x reference)
"""Trainium2 Bass kernel: MultiHeadAttention with relative position embeddings.

Full (unsharded) inputs -> full output. Internally shards 16 heads x 2 batch
across 8 NeuronCores (2 heads/core, both batches on every core); the O-projection
is tensor-parallel over head-dim slices, partials summed on host.
"""
import sys
for p in ("/opt/trn_rl_repo",):
    if p not in sys.path:
        sys.path.append(p)

import numpy as np
import ml_dtypes

import concourse.bass as bass
from concourse import mybir, bacc
from concourse.tile import TileContext
from concourse.masks import make_identity
from concourse.bass_utils import run_bass_kernel_spmd

F16 = mybir.dt.float16
BF16 = mybir.dt.bfloat16
F32 = mybir.dt.float32
NPF16 = np.float16

B, L, D, H, HD = 2, 2048, 1024, 16, 64
MAX_LEN = 2048
NCORES = 8
EPC = 128            # head-dims per core (2 heads x 64)
BL = B * L           # 4096 flattened (b, l)
NT = L // 128        # 16 l-tiles per sequence
BAND = 2176          # QE band width per l-tile (2048 + 127, padded +1)
RELW = 4096          # rel table padded from 4095


def build_bass():
    nc = bacc.Bacc(None)
    xT = nc.declare_dram_parameter("xT", [D, BL], F16, False)
    wq = nc.declare_dram_parameter("wqT8", [D, EPC], F16, False)
    wk = nc.declare_dram_parameter("wkT", [D, EPC], F16, False)
    wv = nc.declare_dram_parameter("wvT", [D, EPC], F16, False)
    wo = nc.declare_dram_parameter("woT", [EPC, D], F16, False)
    rel = nc.declare_dram_parameter("rel8T", [HD, RELW], F16, False)
    bq = nc.declare_dram_parameter("bq8", [EPC, 1], F32, False)
    bk = nc.declare_dram_parameter("bk", [EPC, 1], F32, False)
    bv = nc.declare_dram_parameter("bv", [1, EPC], F16, False)
    bo = nc.declare_dram_parameter("bo", [1, D], F16, False)
    out = nc.declare_dram_parameter("out", [BL, D], F32, True)

    Exp = mybir.ActivationFunctionType.Exp

    with TileContext(nc) as tc:
        with (
            tc.tile_pool(name="singles", bufs=1) as singles,
            tc.tile_pool(name="xin", bufs=8) as xpool,
            tc.tile_pool(name="qe", bufs=3) as qep,
            tc.tile_pool(name="relsh", bufs=3) as relshp,
            tc.tile_pool(name="pp", bufs=2) as ppool,
            tc.tile_pool(name="pt", bufs=3) as ptp,
            tc.tile_pool(name="atts", bufs=4) as attp,
            tc.tile_pool(name="small", bufs=8) as small,
            tc.tile_pool(name="osb", bufs=3) as outp,
            tc.tile_pool(name="scr", bufs=4, space="DRAM") as dramp,
        ):
            # ---- persistent tiles + loads ----
            qT = singles.tile([128, BL], F16, tag="qT")
            kT = singles.tile([128, BL], F16, tag="kT")
            vsb = singles.tile([128, BL // 128, 128], F16, tag="v")
            relsb = singles.tile([128, RELW], F16, tag="rel")
            wosb = singles.tile([128, D], F16, tag="wo")
            ident = singles.tile([128, 128], F16, tag="ident")
            ones1 = singles.tile([1, 128], F16, tag="ones1")
            bq_s = singles.tile([128, 1], F32, tag="bq")
            bk_s = singles.tile([128, 1], F32, tag="bk")
            bv_s = singles.tile([1, 128], F16, tag="bv")
            bo_s = singles.tile([1, D], F16, tag="bo")
            wq_s = singles.tile([128, 8, 128], F16, tag="wq")
            wk_s = singles.tile([128, 8, 128], F16, tag="wk")
            wv_s = singles.tile([128, 8, 128], F16, tag="wv")

            nc.gpsimd.dma_start(out=relsb[0:64, :], in_=rel[:, :])
            nc.gpsimd.dma_start(out=relsb[64:128, :], in_=rel[:, :])
            nc.sync.dma_start(out=wosb, in_=wo[:, :])
            nc.sync.dma_start(out=bq_s, in_=bq[:, :])
            nc.sync.dma_start(out=bk_s, in_=bk[:, :])
            nc.sync.dma_start(out=bv_s, in_=bv[:, :])
            nc.sync.dma_start(out=bo_s, in_=bo[:, :])
            for w_s, w_d in ((wq_s, wq), (wk_s, wk), (wv_s, wv)):
                nc.gpsimd.dma_start(out=w_s, in_=w_d[:, :].rearrange("(k p) e -> p k e", p=128))
            make_identity(nc, ident)
            nc.gpsimd.memset(ones1, 1.0)

            xts = []
            for kk in range(8):
                xt = xpool.tile([128, BL], F16, tag="xt")
                eng = nc.sync if kk < 4 else nc.gpsimd
                eng.dma_start(out=xt, in_=xT[kk * 128:(kk + 1) * 128, :])
                xts.append(xt)

            # ---- phase A: q/k projections (v deferred until after first QE) ----
            with (
                tc.tile_pool(name="psA", bufs=2, space="PSUM") as psA,
                tc.tile_pool(name="psV", bufs=2, space="PSUM") as psV,
            ):
                for nchunk in range(BL // 512):
                    sl = slice(nchunk * 512, (nchunk + 1) * 512)
                    for w_s, b_s, dst in ((wq_s, bq_s, qT), (wk_s, bk_s, kT)):
                        ps = psA.tile([128, 512], F32, tag="psA")
                        for kk in range(8):
                            nc.tensor.matmul(ps, lhsT=w_s[:, kk, :], rhs=xts[kk][:, sl],
                                             start=(kk == 0), stop=(kk == 7))
                        nc.vector.tensor_scalar_add(dst[:, sl], ps, b_s)

                for m in range(BL // 128):
                    ps = psV.tile([128, 128], F32, tag="psV")
                    msl = slice(m * 128, (m + 1) * 128)
                    for kk in range(8):
                        nc.tensor.matmul(ps, lhsT=xts[kk][:, msl], rhs=wv_s[:, kk, :],
                                         start=(kk == 0), stop=False)
                    nc.tensor.matmul(ps, lhsT=ones1, rhs=bv_s, start=False, stop=True)
                    nc.vector.tensor_copy(out=vsb[:, m, :], in_=ps)

            # ---- phase B/C: attention + O-projection, software-pipelined ----
            with (
                tc.tile_pool(name="psS", bufs=2, space="PSUM") as psS,
                tc.tile_pool(name="psQE", bufs=2, space="PSUM") as psQE,
                tc.tile_pool(name="psPT", bufs=2, space="PSUM") as psPT,
                tc.tile_pool(name="psPV", bufs=1, space="PSUM") as psPV,
                tc.tile_pool(name="psO", bufs=1, space="PSUM") as psO,
            ):
                def emit_qe_skew(b, lt):
                    """QE band matmuls + DRAM skew round-trip; returns rsh per head."""
                    pmin = (MAX_LEN - 128) - lt * 128
                    rshs = []
                    for h in range(2):
                        hsl = slice(h * 64, (h + 1) * 64)
                        lq = qT[hsl, b * L + lt * 128: b * L + lt * 128 + 128]
                        qe = qep.tile([128, BAND], F16, tag="qe")
                        for c in range(5):
                            w = 512 if c < 4 else BAND - 4 * 512
                            ps = psQE.tile([128, 512], F32, tag="psQE")
                            nc.tensor.matmul(ps[:, :w], lhsT=lq,
                                             rhs=relsb[hsl, pmin + c * 512: pmin + c * 512 + w],
                                             start=True, stop=True)
                            if c % 2 == 0:
                                nc.vector.tensor_copy(out=qe[:, c * 512: c * 512 + w], in_=ps[:, :w])
                            else:
                                nc.scalar.copy(out=qe[:, c * 512: c * 512 + w], in_=ps[:, :w])
                        scr = dramp.tile([128, BAND], F16, tag="scr")
                        nc.sync.dma_start(out=scr, in_=qe)
                        rsh = relshp.tile([128, L], F16, tag="relsh")
                        nc.gpsimd.dma_start(
                            out=rsh,
                            in_=bass.AP(tensor=scr.tensor, offset=scr.offset + 127,
                                        ap=[[BAND - 1, 128], [1, L]]))
                        rshs.append(rsh)
                    return rshs

                def emit_attention(b, lt, rshs):
                    pvps = psPV.tile([128, 128], F32, tag="pv")
                    pv_first = None
                    for h in range(2):
                        hsl = slice(h * 64, (h + 1) * 64)
                        lq = qT[hsl, b * L + lt * 128: b * L + lt * 128 + 128]
                        rsh = rshs[h]
                        # S = q.k + rel; P = exp(S) with per-chunk Z partials
                        psb = ppool.tile([128, L], BF16, tag="P")
                        z4 = small.tile([128, 4], F32, tag="z4")
                        for c in range(4):
                            csl = slice(c * 512, (c + 1) * 512)
                            sps = psS.tile([128, 512], F32, tag="S")
                            nc.tensor.matmul(sps, lhsT=lq,
                                             rhs=kT[hsl, b * L + c * 512: b * L + (c + 1) * 512],
                                             start=True, stop=False)
                            nc.tensor.matmul(sps, lhsT=ident, rhs=rsh[:, csl],
                                             start=False, stop=True)
                            nc.scalar.activation(out=psb[:, csl], in_=sps, func=Exp,
                                                 accum_out=z4[:, c: c + 1])
                        z = small.tile([128, 1], F32, tag="z")
                        nc.vector.tensor_reduce(out=z, in_=z4, axis=mybir.AxisListType.X,
                                                op=mybir.AluOpType.add)
                        rz = small.tile([128, 1], F32, tag="rz")
                        nc.vector.reciprocal(rz, z)
                        pn = ppool.tile([128, L], F16, tag="Pn")
                        nc.vector.tensor_scalar_mul(pn, psb, rz)
                        # PT = Pn.T (4 transposes per PSUM bank)
                        pt = ptp.tile([128, 16, 128], F16, tag="pt")
                        for g in range(4):
                            tps = psPT.tile([128, 512], F32, tag="ptps")
                            first = None
                            for jj in range(4):
                                j = g * 4 + jj
                                mm = nc.tensor.matmul(
                                    tps[:, jj * 128:(jj + 1) * 128],
                                    lhsT=pn[:, j * 128:(j + 1) * 128],
                                    rhs=ident, start=(jj == 0), stop=(jj == 3),
                                    skip_group_check=True)
                                if jj == 0:
                                    first = mm
                                else:
                                    bass._add_dep_helper(
                                        mm.ins, first.ins, sync=False,
                                        reason="bank-clear transpose first")
                            dst = pt[:, g * 4:(g + 1) * 4, :].rearrange("p a b -> p (a b)")
                            if g < 2:
                                nc.vector.tensor_copy(out=dst, in_=tps)
                            else:
                                nc.scalar.copy(out=dst, in_=tps)
                        # PV accumulation: h0 -> partitions 0:64, h1 -> 64:128.
                        # start=True clears the bank's has_written FLAGS only
                        # (data intact), so h1's group must start after h0's
                        # group fully accumulated.
                        for j in range(16):
                            mm = nc.tensor.matmul(pvps[hsl, :], lhsT=vsb[:, b * 16 + j, hsl],
                                                  rhs=pt[:, j, :],
                                                  start=(j == 0), stop=(j == 15),
                                                  skip_group_check=True)
                            if h == 0:
                                pv_first = mm   # ends as h0's LAST matmul
                            elif j == 0:
                                bass._add_dep_helper(
                                    mm.ins, pv_first.ins, sync=False,
                                    reason="h1 group after h0 group done")
                    att = attp.tile([128, 128], F16, tag="att")
                    nc.vector.tensor_copy(out=att, in_=pvps)
                    # O-projection partial for these 128 rows
                    osb = outp.tile([128, D], F32, tag="osb")
                    for c in range(2):
                        csl = slice(c * 512, (c + 1) * 512)
                        ops = psO.tile([128, 512], F32, tag="psO")
                        nc.tensor.matmul(ops, lhsT=att, rhs=wosb[:, csl],
                                         start=True, stop=False)
                        nc.tensor.matmul(ops, lhsT=ones1, rhs=bo_s[:, csl],
                                         start=False, stop=True)
                        if c == 0:
                            nc.vector.tensor_copy(out=osb[:, csl], in_=ops)
                        else:
                            nc.scalar.copy(out=osb[:, csl], in_=ops)
                    nc.sync.dma_start(out=out[b * L + lt * 128: b * L + lt * 128 + 128, :],
                                      in_=osb)

                order = [(b, lt) for b in range(B) for lt in range(NT)]
                pend = emit_qe_skew(*order[0])
                for i, (b, lt) in enumerate(order):
                    nxt = emit_qe_skew(*order[i + 1]) if i + 1 < len(order) else None
                    emit_attention(b, lt, pend)
                    pend = nxt
    nc.compile()
    return nc


def make_in_maps(inputs):
    x = np.asarray(inputs["x"], np.float32)
    Wq = np.asarray(inputs["Wq"], np.float32)
    bq = np.asarray(inputs["bq"], np.float32)
    Wk = np.asarray(inputs["Wk"], np.float32)
    bk = np.asarray(inputs["bk"], np.float32)
    Wv = np.asarray(inputs["Wv"], np.float32)
    bv = np.asarray(inputs["bv"], np.float32)
    Wo = np.asarray(inputs["Wo"], np.float32)
    bo = np.asarray(inputs["bo"], np.float32)
    rel = np.asarray(inputs["rel_emb"], np.float32)

    s8 = 1.0 / np.sqrt(HD)
    xT = np.ascontiguousarray(x.reshape(BL, D).T).astype(NPF16)
    # rel is scaled by 8 (= 1/s8) because q is pre-scaled by s8
    rel8T = np.zeros((HD, RELW), NPF16)
    rel8T[:, :2 * MAX_LEN - 1] = (rel.T / s8).astype(NPF16)

    in_maps = []
    for c in range(NCORES):
        E = slice(EPC * c, EPC * (c + 1))
        in_maps.append({
            "xT": xT,
            "wqT8": np.ascontiguousarray((Wq[E, :] * s8).T).astype(NPF16),
            "wkT": np.ascontiguousarray(Wk[E, :].T).astype(NPF16),
            "wvT": np.ascontiguousarray(Wv[E, :].T).astype(NPF16),
            "woT": np.ascontiguousarray(Wo[:, E].T).astype(NPF16),
            "rel8T": rel8T,
            "bq8": (bq[E] * s8).astype(np.float32).reshape(EPC, 1),
            "bk": bk[E].astype(np.float32).reshape(EPC, 1),
            "bv": bv[E].astype(NPF16).reshape(1, EPC),
            "bo": (bo if c == 0 else np.zeros_like(bo)).astype(NPF16).reshape(1, D),
        })
    return in_maps


_NC_CACHE = None


def get_nc():
    global _NC_CACHE
    if _NC_CACHE is None:
        _NC_CACHE = build_bass()
    return _NC_CACHE


def kernel(**inputs):
    nc = get_nc()
    in_maps = make_in_maps(inputs)
    res = run_bass_kernel_spmd(nc, in_maps, core_ids=list(range(NCORES)))
    acc = np.zeros((BL, D), np.float64)
    for c in range(NCORES):
        acc += res.results[c]["out"].astype(np.float64)
    return acc.reshape(B, L, D).astype(np.float32)


# revision 18
# speedup vs baseline: 48.9529x; 48.9529x over previous
"""Trainium2 Bass kernel: MultiHeadAttention with relative position embeddings.

Full (unsharded) inputs -> full output. Internally shards 16 heads x 2 batch
across 8 NeuronCores (2 heads/core, both batches on every core); the O-projection
is tensor-parallel over head-dim slices, partials summed on host.
"""
import sys
for p in ("/opt/trn_rl_repo",):
    if p not in sys.path:
        sys.path.append(p)

import numpy as np
import ml_dtypes

import concourse.bass as bass
from concourse import mybir, bacc
from concourse.tile import TileContext
from concourse.masks import make_identity
from concourse.bass_utils import run_bass_kernel_spmd

F16 = mybir.dt.float16
BF16 = mybir.dt.bfloat16
F32 = mybir.dt.float32
NPF16 = np.float16

B, L, D, H, HD = 2, 2048, 1024, 16, 64
MAX_LEN = 2048
NCORES = 8
EPC = 128            # head-dims per core (2 heads x 64)
BL = B * L           # 4096 flattened (b, l)
NT = L // 128        # 16 l-tiles per sequence
BAND = 2176          # QE band width per l-tile (2048 + 127, padded +1)
RELW = 4096          # rel table padded from 4095


def build_bass(replicas=1):
    nc = bacc.Bacc(None)
    xT = nc.declare_dram_parameter("xT", [D, BL], F16, False)
    wq = nc.declare_dram_parameter("wqT8", [D, EPC], F16, False)
    wk = nc.declare_dram_parameter("wkT", [D, EPC], F16, False)
    wv = nc.declare_dram_parameter("wvT", [D, EPC], F16, False)
    wo = nc.declare_dram_parameter("woT", [EPC, D], F16, False)
    rel = nc.declare_dram_parameter("rel8T", [HD, RELW], F16, False)
    bq = nc.declare_dram_parameter("bq8", [EPC, 1], F32, False)
    bk = nc.declare_dram_parameter("bk", [EPC, 1], F32, False)
    bv = nc.declare_dram_parameter("bv", [1, EPC], F16, False)
    bo = nc.declare_dram_parameter("bo", [1, D], F16, False)
    out = nc.declare_dram_parameter("out", [BL, D], F32, True)

    Exp = mybir.ActivationFunctionType.Exp

    with TileContext(nc) as tc:
      for _rep in range(replicas):
        with (
            tc.tile_pool(name="singles", bufs=1) as singles,
            tc.tile_pool(name="xin", bufs=8) as xpool,
            tc.tile_pool(name="qe", bufs=3) as qep,
            tc.tile_pool(name="relsh", bufs=3) as relshp,
            tc.tile_pool(name="pp", bufs=2) as ppool,
            tc.tile_pool(name="pt", bufs=3) as ptp,
            tc.tile_pool(name="atts", bufs=4) as attp,
            tc.tile_pool(name="small", bufs=8) as small,
            tc.tile_pool(name="osb", bufs=3) as outp,
            tc.tile_pool(name="scr", bufs=4, space="DRAM") as dramp,
        ):
            # ---- persistent tiles + loads ----
            qT = singles.tile([128, BL], F16, tag="qT")
            kT = singles.tile([128, BL], F16, tag="kT")
            vsb = singles.tile([128, BL // 128, 128], F16, tag="v")
            relsb = singles.tile([128, RELW], F16, tag="rel")
            wosb = singles.tile([128, D], F16, tag="wo")
            ident = singles.tile([128, 128], F16, tag="ident")
            ones1 = singles.tile([1, 128], F16, tag="ones1")
            bq_s = singles.tile([128, 1], F32, tag="bq")
            bk_s = singles.tile([128, 1], F32, tag="bk")
            bv_s = singles.tile([1, 128], F16, tag="bv")
            bo_s = singles.tile([1, D], F16, tag="bo")
            wq_s = singles.tile([128, 8, 128], F16, tag="wq")
            wk_s = singles.tile([128, 8, 128], F16, tag="wk")
            wv_s = singles.tile([128, 8, 128], F16, tag="wv")

            nc.gpsimd.dma_start(out=relsb[0:64, :], in_=rel[:, :])
            nc.gpsimd.dma_start(out=relsb[64:128, :], in_=rel[:, :])
            nc.sync.dma_start(out=wosb, in_=wo[:, :])
            nc.sync.dma_start(out=bq_s, in_=bq[:, :])
            nc.sync.dma_start(out=bk_s, in_=bk[:, :])
            nc.sync.dma_start(out=bv_s, in_=bv[:, :])
            nc.sync.dma_start(out=bo_s, in_=bo[:, :])
            for w_s, w_d in ((wq_s, wq), (wk_s, wk), (wv_s, wv)):
                nc.gpsimd.dma_start(out=w_s, in_=w_d[:, :].rearrange("(k p) e -> p k e", p=128))
            make_identity(nc, ident)
            nc.gpsimd.memset(ones1, 1.0)

            xts = []
            for kk in range(8):
                xt = xpool.tile([128, BL], F16, tag="xt")
                eng = nc.sync if kk < 4 else nc.gpsimd
                eng.dma_start(out=xt, in_=xT[kk * 128:(kk + 1) * 128, :])
                xts.append(xt)

            # ---- phase A: q/k projections (v deferred until after first QE) ----
            with (
                tc.tile_pool(name="psA", bufs=2, space="PSUM") as psA,
                tc.tile_pool(name="psV", bufs=2, space="PSUM") as psV,
            ):
                for nchunk in range(BL // 512):
                    sl = slice(nchunk * 512, (nchunk + 1) * 512)
                    for w_s, b_s, dst in ((wq_s, bq_s, qT), (wk_s, bk_s, kT)):
                        ps = psA.tile([128, 512], F32, tag="psA")
                        for kk in range(8):
                            nc.tensor.matmul(ps, lhsT=w_s[:, kk, :], rhs=xts[kk][:, sl],
                                             start=(kk == 0), stop=(kk == 7))
                        nc.vector.tensor_scalar_add(dst[:, sl], ps, b_s)

                for m in range(BL // 128):
                    ps = psV.tile([128, 128], F32, tag="psV")
                    msl = slice(m * 128, (m + 1) * 128)
                    for kk in range(8):
                        nc.tensor.matmul(ps, lhsT=xts[kk][:, msl], rhs=wv_s[:, kk, :],
                                         start=(kk == 0), stop=False)
                    nc.tensor.matmul(ps, lhsT=ones1, rhs=bv_s, start=False, stop=True)
                    nc.vector.tensor_copy(out=vsb[:, m, :], in_=ps)

            # ---- phase B/C: attention + O-projection, software-pipelined ----
            with (
                tc.tile_pool(name="psS", bufs=2, space="PSUM") as psS,
                tc.tile_pool(name="psQE", bufs=2, space="PSUM") as psQE,
                tc.tile_pool(name="psPT", bufs=2, space="PSUM") as psPT,
                tc.tile_pool(name="psPV", bufs=1, space="PSUM") as psPV,
                tc.tile_pool(name="psO", bufs=1, space="PSUM") as psO,
            ):
                def emit_qe_skew(b, lt):
                    """QE band matmuls + DRAM skew round-trip; returns rsh per head."""
                    pmin = (MAX_LEN - 128) - lt * 128
                    rshs = []
                    for h in range(2):
                        hsl = slice(h * 64, (h + 1) * 64)
                        lq = qT[hsl, b * L + lt * 128: b * L + lt * 128 + 128]
                        qe = qep.tile([128, BAND], F16, tag="qe")
                        for c in range(5):
                            w = 512 if c < 4 else BAND - 4 * 512
                            ps = psQE.tile([128, 512], F32, tag="psQE")
                            nc.tensor.matmul(ps[:, :w], lhsT=lq,
                                             rhs=relsb[hsl, pmin + c * 512: pmin + c * 512 + w],
                                             start=True, stop=True)
                            if c % 2 == 0:
                                nc.vector.tensor_copy(out=qe[:, c * 512: c * 512 + w], in_=ps[:, :w])
                            else:
                                nc.scalar.copy(out=qe[:, c * 512: c * 512 + w], in_=ps[:, :w])
                        scr = dramp.tile([128, BAND], F16, tag="scr")
                        nc.sync.dma_start(out=scr, in_=qe)
                        rsh = relshp.tile([128, L], F16, tag="relsh")
                        nc.gpsimd.dma_start(
                            out=rsh,
                            in_=bass.AP(tensor=scr.tensor, offset=scr.offset + 127,
                                        ap=[[BAND - 1, 128], [1, L]]))
                        rshs.append(rsh)
                    return rshs

                def emit_attention(b, lt, rshs):
                    pvps = psPV.tile([128, 128], F32, tag="pv")
                    pv_first = None
                    for h in range(2):
                        hsl = slice(h * 64, (h + 1) * 64)
                        lq = qT[hsl, b * L + lt * 128: b * L + lt * 128 + 128]
                        rsh = rshs[h]
                        # S = q.k + rel; P = exp(S) with per-chunk Z partials
                        psb = ppool.tile([128, L], BF16, tag="P")
                        z4 = small.tile([128, 4], F32, tag="z4")
                        for c in range(4):
                            csl = slice(c * 512, (c + 1) * 512)
                            sps = psS.tile([128, 512], F32, tag="S")
                            nc.tensor.matmul(sps, lhsT=lq,
                                             rhs=kT[hsl, b * L + c * 512: b * L + (c + 1) * 512],
                                             start=True, stop=False)
                            nc.tensor.matmul(sps, lhsT=ident, rhs=rsh[:, csl],
                                             start=False, stop=True)
                            nc.scalar.activation(out=psb[:, csl], in_=sps, func=Exp,
                                                 accum_out=z4[:, c: c + 1])
                        z = small.tile([128, 1], F32, tag="z")
                        nc.vector.tensor_reduce(out=z, in_=z4, axis=mybir.AxisListType.X,
                                                op=mybir.AluOpType.add)
                        rz = small.tile([128, 1], F32, tag="rz")
                        nc.vector.reciprocal(rz, z)
                        pn = ppool.tile([128, L], F16, tag="Pn")
                        nc.vector.tensor_scalar_mul(pn, psb, rz)
                        # PT = Pn.T (4 transposes per PSUM bank)
                        pt = ptp.tile([128, 16, 128], F16, tag="pt")
                        for g in range(4):
                            tps = psPT.tile([128, 512], F32, tag="ptps")
                            first = None
                            for jj in range(4):
                                j = g * 4 + jj
                                mm = nc.tensor.matmul(
                                    tps[:, jj * 128:(jj + 1) * 128],
                                    lhsT=pn[:, j * 128:(j + 1) * 128],
                                    rhs=ident, start=(jj == 0), stop=(jj == 3),
                                    skip_group_check=True)
                                if jj == 0:
                                    first = mm
                                else:
                                    bass._add_dep_helper(
                                        mm.ins, first.ins, sync=False,
                                        reason="bank-clear transpose first")
                            dst = pt[:, g * 4:(g + 1) * 4, :].rearrange("p a b -> p (a b)")
                            if g < 2:
                                nc.vector.tensor_copy(out=dst, in_=tps)
                            else:
                                nc.scalar.copy(out=dst, in_=tps)
                        # PV accumulation: h0 -> partitions 0:64, h1 -> 64:128.
                        # start=True clears the bank's has_written FLAGS only
                        # (data intact), so h1's group must start after h0's
                        # group fully accumulated.
                        for j in range(16):
                            mm = nc.tensor.matmul(pvps[hsl, :], lhsT=vsb[:, b * 16 + j, hsl],
                                                  rhs=pt[:, j, :],
                                                  start=(j == 0), stop=(j == 15),
                                                  skip_group_check=True)
                            if h == 0:
                                pv_first = mm   # ends as h0's LAST matmul
                            elif j == 0:
                                bass._add_dep_helper(
                                    mm.ins, pv_first.ins, sync=False,
                                    reason="h1 group after h0 group done")
                    att = attp.tile([128, 128], F16, tag="att")
                    nc.vector.tensor_copy(out=att, in_=pvps)
                    # O-projection partial for these 128 rows
                    osb = outp.tile([128, D], F32, tag="osb")
                    for c in range(2):
                        csl = slice(c * 512, (c + 1) * 512)
                        ops = psO.tile([128, 512], F32, tag="psO")
                        nc.tensor.matmul(ops, lhsT=att, rhs=wosb[:, csl],
                                         start=True, stop=False)
                        nc.tensor.matmul(ops, lhsT=ones1, rhs=bo_s[:, csl],
                                         start=False, stop=True)
                        if c == 0:
                            nc.vector.tensor_copy(out=osb[:, csl], in_=ops)
                        else:
                            nc.scalar.copy(out=osb[:, csl], in_=ops)
                    nc.sync.dma_start(out=out[b * L + lt * 128: b * L + lt * 128 + 128, :],
                                      in_=osb)

                order = [(b, lt) for b in range(B) for lt in range(NT)]
                pend = emit_qe_skew(*order[0])
                for i, (b, lt) in enumerate(order):
                    nxt = emit_qe_skew(*order[i + 1]) if i + 1 < len(order) else None
                    emit_attention(b, lt, pend)
                    pend = nxt
    nc.compile()
    return nc


def make_in_maps(inputs):
    x = np.asarray(inputs["x"], np.float32)
    Wq = np.asarray(inputs["Wq"], np.float32)
    bq = np.asarray(inputs["bq"], np.float32)
    Wk = np.asarray(inputs["Wk"], np.float32)
    bk = np.asarray(inputs["bk"], np.float32)
    Wv = np.asarray(inputs["Wv"], np.float32)
    bv = np.asarray(inputs["bv"], np.float32)
    Wo = np.asarray(inputs["Wo"], np.float32)
    bo = np.asarray(inputs["bo"], np.float32)
    rel = np.asarray(inputs["rel_emb"], np.float32)

    s8 = 1.0 / np.sqrt(HD)
    xT = np.ascontiguousarray(x.reshape(BL, D).T).astype(NPF16)
    # rel is scaled by 8 (= 1/s8) because q is pre-scaled by s8
    rel8T = np.zeros((HD, RELW), NPF16)
    rel8T[:, :2 * MAX_LEN - 1] = (rel.T / s8).astype(NPF16)

    in_maps = []
    for c in range(NCORES):
        E = slice(EPC * c, EPC * (c + 1))
        in_maps.append({
            "xT": xT,
            "wqT8": np.ascontiguousarray((Wq[E, :] * s8).T).astype(NPF16),
            "wkT": np.ascontiguousarray(Wk[E, :].T).astype(NPF16),
            "wvT": np.ascontiguousarray(Wv[E, :].T).astype(NPF16),
            "woT": np.ascontiguousarray(Wo[:, E].T).astype(NPF16),
            "rel8T": rel8T,
            "bq8": (bq[E] * s8).astype(np.float32).reshape(EPC, 1),
            "bk": bk[E].astype(np.float32).reshape(EPC, 1),
            "bv": bv[E].astype(NPF16).reshape(1, EPC),
            "bo": (bo if c == 0 else np.zeros_like(bo)).astype(NPF16).reshape(1, D),
        })
    return in_maps


_NC_CACHE = None


def get_nc():
    global _NC_CACHE
    if _NC_CACHE is None:
        _NC_CACHE = build_bass()
    return _NC_CACHE


def kernel(**inputs):
    nc = get_nc()
    in_maps = make_in_maps(inputs)
    res = run_bass_kernel_spmd(nc, in_maps, core_ids=list(range(NCORES)))
    acc = np.zeros((BL, D), np.float64)
    for c in range(NCORES):
        acc += res.results[c]["out"].astype(np.float64)
    return acc.reshape(B, L, D).astype(np.float32)
